# revision 19
# baseline (speedup 1.0000x reference)
"""Causal self-attention (B=4, S=2048, D=1024, single head) on 8 TRN2 cores.

Sharding: data-parallel over batch (4 batches x 2 cores). The two cores of a
batch split the 16 query tiles (128 rows each) so both get exactly equal
causal work: core A takes tiles {0,2,4,6,9,11,13,15}, core B the complement.
The shared SPMD program processes NS[s] key-tiles for slot s (max over the
two variants); which global query tile a slot holds is carried by the input
data (host-side slicing + an additive causal mask input for the diagonal
512-chunk of each slot; earlier chunks are fully valid for both variants).

Per core: one pass over x^T strips projects Q^T, K^T (e-major) and V
(k-major), all resident in SBUF. Then per query-tile slot: scores on PE,
exp straight out of PSUM on ACT with per-chunk accumulated row sums (the
diagonal chunk takes a DVE mask-add first), PE-transpose of the
probabilities, PV accumulation on PE, normalize by the reciprocal row-sum.

Modes (BASS_KERNEL_MODE env, default "bf16"):
  "bf16" — inputs cast to bf16 host-side; ~230us, rel err 6.0e-3.
  "f32r" — fp32 storage, reduced-precision PE multiply; ~301us, rel err 3.0e-4.
  "cc"   — bf16 + intra-pair AllGather K/V dedup. DO NOT USE: collectives
           hang under the axon/PJRT execution path in this container.
"""

import os
from contextlib import ExitStack

import ml_dtypes
import numpy as np

import concourse.bacc as bacc
import concourse.mybir as mybir
import concourse.tile as tile
from concourse.bass_utils import run_bass_kernel_spmd

B, S, D = 4, 2048, 1024
P = 128
DC = D // P  # 8 contraction chunks
A_TILES = [0, 2, 4, 6, 9, 11, 13, 15]
B_TILES = [1, 3, 5, 7, 8, 10, 12, 14]
NS = [2, 4, 6, 8, 10, 12, 14, 16]  # key-tiles (128 keys) processed per slot
MASK_VAL = -60.0

F32 = mybir.dt.float32
F32R = mybir.dt.float32r
BF16 = mybir.dt.bfloat16

MODE = os.environ.get("BASS_KERNEL_MODE", "bf16")

_compiled = {}


def _build_bf16():
    nc = bacc.Bacc("TRN2", target_bir_lowering=False, debug=False)
    xqT = nc.dram_tensor("xqT", [D, 1024], BF16, kind="ExternalInput").ap()
    xT = nc.dram_tensor("xT", [D, S], BF16, kind="ExternalInput").ap()
    wq = nc.dram_tensor("wq", [D, D], BF16, kind="ExternalInput").ap()
    wk = nc.dram_tensor("wk", [D, D], BF16, kind="ExternalInput").ap()
    wv = nc.dram_tensor("wv", [D, D], BF16, kind="ExternalInput").ap()
    msk = nc.dram_tensor("msk", [1024, 512], F32, kind="ExternalInput").ap()
    out_d = nc.dram_tensor("out", [1024, D], F32, kind="ExternalOutput").ap()

    with tile.TileContext(nc) as tc:
        _body_bf16(tc, xqT, xT, wq, wk, wv, msk, out_d)
    nc.compile()
    return nc


def _body_bf16(tc, xqT, xT, wq, wk, wv, msk, out_d):
    nc = tc.nc
    with ExitStack() as top:
        const_pool = top.enter_context(tc.tile_pool(name="cst", bufs=1))
        ident = const_pool.tile([P, P], BF16, name="ident", tag="ident")
        nc.gpsimd.memset(ident[:], 1.0)
        nc.gpsimd.affine_select(
            out=ident[:],
            in_=ident[:],
            compare_op=mybir.AluOpType.is_equal,
            fill=0.0,
            base=0,
            pattern=[[-1, P]],
            channel_multiplier=1,
        )

        # resident tensors
        res_pool = top.enter_context(tc.tile_pool(name="res", bufs=1))
        QT = [res_pool.tile([P, 1024], BF16, name=f"qt{e}", tag=f"qt{e}") for e in range(DC)]
        KT = [res_pool.tile([P, S], BF16, name=f"kt{e}", tag=f"kt{e}") for e in range(DC)]
        V = [res_pool.tile([P, D], BF16, name=f"v{k}", tag=f"v{k}") for k in range(S // P)]

        # weights resident (bf16: 3 * 16KB/partition)
        w_pool = top.enter_context(tc.tile_pool(name="wp", bufs=1))
        wq_t = [w_pool.tile([P, D], BF16, name=f"wqt{d}", tag=f"wqt{d}") for d in range(DC)]
        wk_t = [w_pool.tile([P, D], BF16, name=f"wkt{d}", tag=f"wkt{d}") for d in range(DC)]
        wv_t = [w_pool.tile([P, D], BF16, name=f"wvt{d}", tag=f"wvt{d}") for d in range(DC)]

        # one PSUM pool for the whole kernel (8 banks: 2+2+2+2)
        psum = top.enter_context(tc.tile_pool(name="psum", bufs=2, space="PSUM"))
        # streaming x strips
        xs_pool = top.enter_context(tc.tile_pool(name="xsp", bufs=2))

        # ---------------- K + V projections (single strip pass) ----------------
        # V runs first within each strip: its first matmul group is gated on
        # only strip0 + the ec0 half of Wv (2MB), vs 3MB for a W-stationary
        # first group — PE starts earlier. Q runs last (attention needs Q^T
        # no sooner than the end of the projections anyway).
        # Wv ec0-half first, then strip0 in column-chunks: with subtile dep
        # tracking the first V matmul group is gated on ~1.25MB, not 2MB.
        for d in range(DC):
            nc.sync.dma_start(wv_t[d][:, 0:512], wv[d * P : (d + 1) * P, 0:512])
        xs0 = [xs_pool.tile([P, 512], BF16, name=f"xs{d}", tag=f"xs{d}") for d in range(DC)]
        for col in range(4):
            for d in range(DC):
                nc.sync.dma_start(
                    xs0[d][:, col * P : (col + 1) * P],
                    xT[d * P : (d + 1) * P, col * P : (col + 1) * P],
                )
        for d in range(DC):
            nc.sync.dma_start(wv_t[d][:, 512:1024], wv[d * P : (d + 1) * P, 512:1024])
        for d in range(DC):
            nc.sync.dma_start(wk_t[d][:], wk[d * P : (d + 1) * P, :])
        for kc in range(4):
            if kc == 0:
                xs = xs0
            else:
                xs = [xs_pool.tile([P, 512], BF16, name=f"xs{d}", tag=f"xs{d}") for d in range(DC)]
                for d in range(DC):
                    nc.sync.dma_start(xs[d][:], xT[d * P : (d + 1) * P, kc * 512 : (kc + 1) * 512])
            for ec in range(2):
                for j in range(4):
                    kt_idx = kc * 4 + j
                    ps = psum.tile([P, 512], F32, name="pps", tag="pps")
                    for d in range(DC):
                        nc.tensor.matmul(
                            ps[:],
                            lhsT=xs[d][:, j * P : (j + 1) * P],
                            rhs=wv_t[d][:, ec * 512 : (ec + 1) * 512],
                            start=(d == 0),
                            stop=(d == DC - 1),
                        )
                    nc.scalar.copy(V[kt_idx][:, ec * 512 : (ec + 1) * 512], ps[:])
            for e in range(DC):
                ps = psum.tile([P, 512], F32, name="pps", tag="pps")
                for d in range(DC):
                    nc.tensor.matmul(
                        ps[:],
                        lhsT=wk_t[d][:, e * P : (e + 1) * P],
                        rhs=xs[d][:],
                        start=(d == 0),
                        stop=(d == DC - 1),
                    )
                nc.scalar.copy(KT[e][:, kc * 512 : (kc + 1) * 512], ps[:])

        # ---------------- Q projection (last) ----------------
        for d in range(DC):
            nc.sync.dma_start(wq_t[d][:], wq[d * P : (d + 1) * P, :])
        for qc in range(2):
            xs = [xs_pool.tile([P, 512], BF16, name=f"xs{d}", tag=f"xs{d}") for d in range(DC)]
            for d in range(DC):
                nc.sync.dma_start(xs[d][:], xqT[d * P : (d + 1) * P, qc * 512 : (qc + 1) * 512])
            for e in range(DC):
                ps = psum.tile([P, 512], F32, name="pps", tag="pps")
                for d in range(DC):
                    nc.tensor.matmul(
                        ps[:],
                        lhsT=wq_t[d][:, e * P : (e + 1) * P],
                        rhs=xs[d][:],
                        start=(d == 0),
                        stop=(d == DC - 1),
                    )
                nc.scalar.copy(QT[e][:, qc * 512 : (qc + 1) * 512], ps[:])

        # ---------------- attention over slots ----------------
        with ExitStack() as ph:
            m_pool = ph.enter_context(tc.tile_pool(name="mp", bufs=2))
            sm_pool = ph.enter_context(tc.tile_pool(name="smp", bufs=2))
            p_pool = ph.enter_context(tc.tile_pool(name="pp", bufs=2))
            pt_pool = ph.enter_context(tc.tile_pool(name="ptp", bufs=2))
            o_pool = ph.enter_context(tc.tile_pool(name="op", bufs=2))
            small_pool = ph.enter_context(tc.tile_pool(name="smallp", bufs=2))

            for s in range(8):
                n = NS[s]
                kw = n * P
                nchunks = (kw + 511) // 512
                mt = m_pool.tile([P, 512], F32, name="mt", tag="mt")
                last_nw = kw - (nchunks - 1) * 512
                nc.sync.dma_start(mt[:, :last_nw], msk[s * P : (s + 1) * P, :last_nw])
                pb = p_pool.tile([P, S], BF16, name="pb", tag="pb")
                lparts = small_pool.tile([P, 4], F32, name="lparts", tag="lparts")
                for c in range(nchunks):
                    nw = min(512, kw - c * 512)
                    ps = psum.tile([P, 512], F32, name="qk_t", tag="qk_t")
                    for e in range(DC):
                        nc.tensor.matmul(
                            ps[:, :nw],
                            lhsT=QT[e][:, s * P : (s + 1) * P],
                            rhs=KT[e][:, c * 512 : c * 512 + nw],
                            start=(e == 0),
                            stop=(e == DC - 1),
                        )
                    if c == nchunks - 1:
                        sm = sm_pool.tile([P, 512], F32, name="sm", tag="sm")
                        nc.vector.tensor_add(sm[:, :nw], ps[:, :nw], mt[:, :nw])
                        src = sm[:, :nw]
                    else:
                        src = ps[:, :nw]
                    nc.scalar.activation(
                        pb[:, c * 512 : c * 512 + nw],
                        src,
                        mybir.ActivationFunctionType.Exp,
                        accum_out=lparts[:, c : c + 1],
                    )
                lsum = small_pool.tile([P, 1], F32, name="lsum", tag="lsum")
                nc.vector.reduce_sum(lsum[:], lparts[:, :nchunks], axis=mybir.AxisListType.X)
                pt = pt_pool.tile([P, S], BF16, name="ptt", tag="ptt")
                for j in range(n):
                    tps = psum.tile([P, P], BF16, name="tps_t", tag="tps_t")
                    nc.tensor.transpose(tps[:], pb[:, j * P : (j + 1) * P], ident[:])
                    nc.vector.tensor_copy(pt[:, j * P : (j + 1) * P], tps[:])
                rl = small_pool.tile([P, 1], F32, name="rl", tag="rl")
                nc.vector.reciprocal(rl[:], lsum[:])
                ot = o_pool.tile([P, D], F32, name="ot", tag="ot")
                for ec in range(2):
                    ops = psum.tile([P, 512], F32, name="o_t", tag="o_t")
                    for j in range(n):
                        nc.tensor.matmul(
                            ops[:],
                            lhsT=pt[:, j * P : (j + 1) * P],
                            rhs=V[j][:, ec * 512 : (ec + 1) * 512],
                            start=(j == 0),
                            stop=(j == n - 1),
                        )
                    nc.vector.tensor_scalar_mul(ot[:, ec * 512 : (ec + 1) * 512], ops[:], rl[:])
                    nc.sync.dma_start(
                        out_d[s * P : (s + 1) * P, ec * 512 : (ec + 1) * 512],
                        ot[:, ec * 512 : (ec + 1) * 512],
                    )


# ---------------------------------------------------------------------------
# bf16 + collective K/V projection dedup: each core projects only its key-half
# of K^T and V; an intra-pair AllGather (ranks {2b, 2b+1}) rebuilds the full
# tensors. Saves 256 of 640 projection matmuls per core.
# ---------------------------------------------------------------------------


def _build_bf16_cc():
    nc = bacc.Bacc("TRN2", target_bir_lowering=False, debug=False)
    xqT = nc.dram_tensor("xqT", [D, 1024], BF16, kind="ExternalInput").ap()
    xTh = nc.dram_tensor("xTh", [D, 1024], BF16, kind="ExternalInput").ap()
    wq = nc.dram_tensor("wq", [D, D], BF16, kind="ExternalInput").ap()
    wk = nc.dram_tensor("wk", [D, D], BF16, kind="ExternalInput").ap()
    wv = nc.dram_tensor("wv", [D, D], BF16, kind="ExternalInput").ap()
    msk = nc.dram_tensor("msk", [1024, 512], F32, kind="ExternalInput").ap()
    out_d = nc.dram_tensor("out", [1024, D], F32, kind="ExternalOutput").ap()

    with tile.TileContext(nc) as tc:
        _body_bf16_cc(tc, xqT, xTh, wq, wk, wv, msk, out_d)
    nc.compile()
    return nc


def _body_bf16_cc(tc, xqT, xTh, wq, wk, wv, msk, out_d):
    nc = tc.nc
    RG = [[0, 1], [2, 3], [4, 5], [6, 7]]
    with ExitStack() as top:
        const_pool = top.enter_context(tc.tile_pool(name="cst", bufs=1))
        ident = const_pool.tile([P, P], BF16, name="ident", tag="ident")
        nc.gpsimd.memset(ident[:], 1.0)
        nc.gpsimd.affine_select(
            out=ident[:],
            in_=ident[:],
            compare_op=mybir.AluOpType.is_equal,
            fill=0.0,
            base=0,
            pattern=[[-1, P]],
            channel_multiplier=1,
        )

        res_pool = top.enter_context(tc.tile_pool(name="res", bufs=1))
        QT = [res_pool.tile([P, 1024], BF16, name=f"qt{e}", tag=f"qt{e}") for e in range(DC)]
        KT = [res_pool.tile([P, S], BF16, name=f"kt{e}", tag=f"kt{e}") for e in range(DC)]
        V = [res_pool.tile([P, D], BF16, name=f"v{k}", tag=f"v{k}") for k in range(S // P)]

        w_pool = top.enter_context(tc.tile_pool(name="wp", bufs=1))
        wq_t = [w_pool.tile([P, D], BF16, name=f"wqt{d}", tag=f"wqt{d}") for d in range(DC)]
        wk_t = [w_pool.tile([P, D], BF16, name=f"wkt{d}", tag=f"wkt{d}") for d in range(DC)]
        wv_t = [w_pool.tile([P, D], BF16, name=f"wvt{d}", tag=f"wvt{d}") for d in range(DC)]
        for d in range(DC):
            nc.sync.dma_start(wk_t[d][:], wk[d * P : (d + 1) * P, :])
            nc.sync.dma_start(wv_t[d][:], wv[d * P : (d + 1) * P, :])

        dram_pool = top.enter_context(tc.tile_pool(name="ccb", bufs=1, space="DRAM"))
        k_in = dram_pool.tile([1024, 1024], BF16, name="k_in", tag="k_in")
        k_out = dram_pool.tile([2048, 1024], BF16, name="k_out", tag="k_out")
        v_in = dram_pool.tile([1024, 1024], BF16, name="v_in", tag="v_in")
        v_out = dram_pool.tile([2048, 1024], BF16, name="v_out", tag="v_out")

        psum = top.enter_context(tc.tile_pool(name="psum", bufs=2, space="PSUM"))
        xs_pool = top.enter_context(tc.tile_pool(name="xsp", bufs=2))
        stage_pool = top.enter_context(tc.tile_pool(name="stg", bufs=4))

        # ---------------- K half projection -> k_in ----------------
        for kc in range(2):
            xs = [xs_pool.tile([P, 512], BF16, name=f"xs{d}", tag=f"xs{d}") for d in range(DC)]
            for d in range(DC):
                nc.sync.dma_start(xs[d][:], xTh[d * P : (d + 1) * P, kc * 512 : (kc + 1) * 512])
            for e in range(DC):
                ps = psum.tile([P, 512], F32, name="pps", tag="pps")
                for d in range(DC):
                    nc.tensor.matmul(
                        ps[:],
                        lhsT=wk_t[d][:, e * P : (e + 1) * P],
                        rhs=xs[d][:],
                        start=(d == 0),
                        stop=(d == DC - 1),
                    )
                sb = stage_pool.tile([P, 512], BF16, name="ksb", tag="ksb")
                nc.scalar.copy(sb[:], ps[:])
                nc.sync.dma_start(k_in[e * P : (e + 1) * P, kc * 512 : (kc + 1) * 512], sb[:])
        nc.gpsimd.collective_compute(
            "AllGather",
            mybir.AluOpType.bypass,
            replica_groups=RG,
            ins=[k_in[:]],
            outs=[k_out[:]],
        )

        # ---------------- V half projection -> v_in ----------------
        for kc in range(2):
            xs = [xs_pool.tile([P, 512], BF16, name=f"xs{d}", tag=f"xs{d}") for d in range(DC)]
            for d in range(DC):
                nc.sync.dma_start(xs[d][:], xTh[d * P : (d + 1) * P, kc * 512 : (kc + 1) * 512])
            for j in range(4):
                ktl = kc * 4 + j
                for ec in range(2):
                    ps = psum.tile([P, 512], F32, name="pps", tag="pps")
                    for d in range(DC):
                        nc.tensor.matmul(
                            ps[:],
                            lhsT=xs[d][:, j * P : (j + 1) * P],
                            rhs=wv_t[d][:, ec * 512 : (ec + 1) * 512],
                            start=(d == 0),
                            stop=(d == DC - 1),
                        )
                    sb = stage_pool.tile([P, 512], BF16, name="vsb", tag="vsb")
                    nc.scalar.copy(sb[:], ps[:])
                    nc.sync.dma_start(v_in[ktl * P : (ktl + 1) * P, ec * 512 : (ec + 1) * 512], sb[:])
        nc.gpsimd.collective_compute(
            "AllGather",
            mybir.AluOpType.bypass,
            replica_groups=RG,
            ins=[v_in[:]],
            outs=[v_out[:]],
        )

        # ---------------- Q projection (local, no comm) ----------------
        for d in range(DC):
            nc.sync.dma_start(wq_t[d][:], wq[d * P : (d + 1) * P, :])
        for qc in range(2):
            xs = [xs_pool.tile([P, 512], BF16, name=f"xs{d}", tag=f"xs{d}") for d in range(DC)]
            for d in range(DC):
                nc.sync.dma_start(xs[d][:], xqT[d * P : (d + 1) * P, qc * 512 : (qc + 1) * 512])
            for e in range(DC):
                ps = psum.tile([P, 512], F32, name="pps", tag="pps")
                for d in range(DC):
                    nc.tensor.matmul(
                        ps[:],
                        lhsT=wq_t[d][:, e * P : (e + 1) * P],
                        rhs=xs[d][:],
                        start=(d == 0),
                        stop=(d == DC - 1),
                    )
                nc.scalar.copy(QT[e][:, qc * 512 : (qc + 1) * 512], ps[:])

        # ---------------- gather results -> SBUF residents ----------------
        for e in range(DC):
            nc.sync.dma_start(KT[e][:, 0:1024], k_out[e * P : (e + 1) * P, :])
            nc.sync.dma_start(KT[e][:, 1024:2048], k_out[1024 + e * P : 1024 + (e + 1) * P, :])
        for kt in range(S // P):
            nc.sync.dma_start(V[kt][:], v_out[kt * P : (kt + 1) * P, :])

        # ---------------- attention over slots ----------------
        with ExitStack() as ph:
            m_pool = ph.enter_context(tc.tile_pool(name="mp", bufs=2))
            sm_pool = ph.enter_context(tc.tile_pool(name="smp", bufs=2))
            p_pool = ph.enter_context(tc.tile_pool(name="pp", bufs=2))
            pt_pool = ph.enter_context(tc.tile_pool(name="ptp", bufs=2))
            o_pool = ph.enter_context(tc.tile_pool(name="op", bufs=2))
            small_pool = ph.enter_context(tc.tile_pool(name="smallp", bufs=2))

            for s in range(8):
                n = NS[s]
                kw = n * P
                nchunks = (kw + 511) // 512
                mt = m_pool.tile([P, 512], F32, name="mt", tag="mt")
                last_nw = kw - (nchunks - 1) * 512
                nc.sync.dma_start(mt[:, :last_nw], msk[s * P : (s + 1) * P, :last_nw])
                pb = p_pool.tile([P, S], BF16, name="pb", tag="pb")
                lparts = small_pool.tile([P, 4], F32, name="lparts", tag="lparts")
                for c in range(nchunks):
                    nw = min(512, kw - c * 512)
                    ps = psum.tile([P, 512], F32, name="qk_t", tag="qk_t")
                    for e in range(DC):
                        nc.tensor.matmul(
                            ps[:, :nw],
                            lhsT=QT[e][:, s * P : (s + 1) * P],
                            rhs=KT[e][:, c * 512 : c * 512 + nw],
                            start=(e == 0),
                            stop=(e == DC - 1),
                        )
                    if c == nchunks - 1:
                        sm = sm_pool.tile([P, 512], F32, name="sm", tag="sm")
                        nc.vector.tensor_add(sm[:, :nw], ps[:, :nw], mt[:, :nw])
                        src = sm[:, :nw]
                    else:
                        src = ps[:, :nw]
                    nc.scalar.activation(
                        pb[:, c * 512 : c * 512 + nw],
                        src,
                        mybir.ActivationFunctionType.Exp,
                        accum_out=lparts[:, c : c + 1],
                    )
                lsum = small_pool.tile([P, 1], F32, name="lsum", tag="lsum")
                nc.vector.reduce_sum(lsum[:], lparts[:, :nchunks], axis=mybir.AxisListType.X)
                pt = pt_pool.tile([P, S], BF16, name="ptt", tag="ptt")
                for j in range(n):
                    tps = psum.tile([P, P], BF16, name="tps_t", tag="tps_t")
                    nc.tensor.transpose(tps[:], pb[:, j * P : (j + 1) * P], ident[:])
                    nc.vector.tensor_copy(pt[:, j * P : (j + 1) * P], tps[:])
                rl = small_pool.tile([P, 1], F32, name="rl", tag="rl")
                nc.vector.reciprocal(rl[:], lsum[:])
                ot = o_pool.tile([P, D], F32, name="ot", tag="ot")
                for ec in range(2):
                    ops = psum.tile([P, 512], F32, name="o_t", tag="o_t")
                    for j in range(n):
                        nc.tensor.matmul(
                            ops[:],
                            lhsT=pt[:, j * P : (j + 1) * P],
                            rhs=V[j][:, ec * 512 : (ec + 1) * 512],
                            start=(j == 0),
                            stop=(j == n - 1),
                        )
                    nc.vector.tensor_scalar_mul(ot[:, ec * 512 : (ec + 1) * 512], ops[:], rl[:])
                    nc.sync.dma_start(
                        out_d[s * P : (s + 1) * P, ec * 512 : (ec + 1) * 512],
                        ot[:, ec * 512 : (ec + 1) * 512],
                    )


# ---------------------------------------------------------------------------
# f32r fallback (fp32 storage, reduced-precision multiply; Q^T spilled to DRAM)
# ---------------------------------------------------------------------------


def _build_f32r():
    nc = bacc.Bacc("TRN2", target_bir_lowering=False, debug=False)
    xqT = nc.dram_tensor("xqT", [D, 1024], F32R, kind="ExternalInput").ap()
    xT = nc.dram_tensor("xT", [D, S], F32R, kind="ExternalInput").ap()
    wq = nc.dram_tensor("wq", [D, D], F32R, kind="ExternalInput").ap()
    wk = nc.dram_tensor("wk", [D, D], F32R, kind="ExternalInput").ap()
    wv = nc.dram_tensor("wv", [D, D], F32R, kind="ExternalInput").ap()
    msk = nc.dram_tensor("msk", [1024, 512], F32, kind="ExternalInput").ap()
    out_d = nc.dram_tensor("out", [1024, D], F32, kind="ExternalOutput").ap()
    qt_dram = nc.dram_tensor("qt_spill", [D, 1024], F32R).ap()

    with tile.TileContext(nc) as tc:
        _body_f32r(tc, xqT, xT, wq, wk, wv, msk, out_d, qt_dram)
    nc.compile()
    return nc


def _body_f32r(tc, xqT, xT, wq, wk, wv, msk, out_d, qt_dram):
    nc = tc.nc
    with ExitStack() as top:
        const_pool = top.enter_context(tc.tile_pool(name="cst", bufs=1))
        ident = const_pool.tile([P, P], F32, name="ident", tag="ident")
        nc.gpsimd.memset(ident[:], 1.0)
        nc.gpsimd.affine_select(
            out=ident[:],
            in_=ident[:],
            compare_op=mybir.AluOpType.is_equal,
            fill=0.0,
            base=0,
            pattern=[[-1, P]],
            channel_multiplier=1,
        )

        # ---------------- Q projection: Q^T -> DRAM spill ----------------
        with ExitStack() as ph:
            xq_pool = ph.enter_context(tc.tile_pool(name="xqp", bufs=1))
            wq_pool = ph.enter_context(tc.tile_pool(name="wqp", bufs=1))
            qps_pool = ph.enter_context(tc.tile_pool(name="qps", bufs=3, space="PSUM"))
            qsb_pool = ph.enter_context(tc.tile_pool(name="qsb", bufs=3))
            xq_t = [xq_pool.tile([P, 1024], F32R, name=f"xq{d}", tag=f"xq{d}") for d in range(DC)]
            wq_t = [wq_pool.tile([P, D], F32R, name=f"wqt{d}", tag=f"wqt{d}") for d in range(DC)]
            for d in range(DC):
                nc.sync.dma_start(xq_t[d][:], xqT[d * P : (d + 1) * P, :])
                nc.sync.dma_start(wq_t[d][:], wq[d * P : (d + 1) * P, :])
            for e in range(DC):
                for qc in range(2):
                    ps = qps_pool.tile([P, 512], F32, name="qps_t", tag="qps_t")
                    for d in range(DC):
                        nc.tensor.matmul(
                            ps[:],
                            lhsT=wq_t[d][:, e * P : (e + 1) * P],
                            rhs=xq_t[d][:, qc * 512 : (qc + 1) * 512],
                            start=(d == 0),
                            stop=(d == DC - 1),
                        )
                    sb = qsb_pool.tile([P, 512], F32R, name="qsb_t", tag="qsb_t")
                    nc.scalar.copy(sb[:], ps[:])
                    nc.sync.dma_start(
                        qt_dram[e * P : (e + 1) * P, qc * 512 : (qc + 1) * 512], sb[:]
                    )

        # ---------------- K projection: K^T resident ----------------
        kt_pool = top.enter_context(tc.tile_pool(name="ktp", bufs=1))
        KT = [kt_pool.tile([P, S], F32R, name=f"kt{e}", tag=f"kt{e}") for e in range(DC)]
        with ExitStack() as ph:
            wk_pool = ph.enter_context(tc.tile_pool(name="wkp", bufs=1))
            xs_pool = ph.enter_context(tc.tile_pool(name="xsp", bufs=2))
            kps_pool = ph.enter_context(tc.tile_pool(name="kps", bufs=3, space="PSUM"))
            wk_t = [wk_pool.tile([P, D], F32R, name=f"wkt{d}", tag=f"wkt{d}") for d in range(DC)]
            for d in range(DC):
                nc.sync.dma_start(wk_t[d][:], wk[d * P : (d + 1) * P, :])
            for kc in range(4):
                xs = [xs_pool.tile([P, 512], F32R, name=f"xs{d}", tag=f"xs{d}") for d in range(DC)]
                for d in range(DC):
                    nc.sync.dma_start(xs[d][:], xT[d * P : (d + 1) * P, kc * 512 : (kc + 1) * 512])
                for e in range(DC):
                    ps = kps_pool.tile([P, 512], F32, name="kps_t", tag="kps_t")
                    for d in range(DC):
                        nc.tensor.matmul(
                            ps[:],
                            lhsT=wk_t[d][:, e * P : (e + 1) * P],
                            rhs=xs[d][:],
                            start=(d == 0),
                            stop=(d == DC - 1),
                        )
                    nc.scalar.copy(KT[e][:, kc * 512 : (kc + 1) * 512], ps[:])

        # ---------------- V projection: V resident ----------------
        v_pool = top.enter_context(tc.tile_pool(name="vp", bufs=1))
        V = [v_pool.tile([P, D], F32R, name=f"v{k}", tag=f"v{k}") for k in range(S // P)]
        with ExitStack() as ph:
            wv_pool = ph.enter_context(tc.tile_pool(name="wvp", bufs=1))
            xsv_pool = ph.enter_context(tc.tile_pool(name="xsvp", bufs=2))
            vps_pool = ph.enter_context(tc.tile_pool(name="vps", bufs=3, space="PSUM"))
            for ec in range(2):
                wv_t = [wv_pool.tile([P, 512], F32R, name=f"wvt{d}", tag=f"wvt{d}") for d in range(DC)]
                for d in range(DC):
                    nc.sync.dma_start(wv_t[d][:], wv[d * P : (d + 1) * P, ec * 512 : (ec + 1) * 512])
                for kc in range(4):
                    xs = [xsv_pool.tile([P, 512], F32R, name=f"xsv{d}", tag=f"xsv{d}") for d in range(DC)]
                    for d in range(DC):
                        nc.sync.dma_start(
                            xs[d][:], xT[d * P : (d + 1) * P, kc * 512 : (kc + 1) * 512]
                        )
                    for j in range(4):
                        kt_idx = kc * 4 + j
                        ps = vps_pool.tile([P, 512], F32, name="vps_t", tag="vps_t")
                        for d in range(DC):
                            nc.tensor.matmul(
                                ps[:],
                                lhsT=xs[d][:, j * P : (j + 1) * P],
                                rhs=wv_t[d][:],
                                start=(d == 0),
                                stop=(d == DC - 1),
                            )
                        nc.scalar.copy(V[kt_idx][:, ec * 512 : (ec + 1) * 512], ps[:])

        # ---------------- attention over slots ----------------
        with ExitStack() as ph:
            qts_pool = ph.enter_context(tc.tile_pool(name="qtsp", bufs=2))
            m_pool = ph.enter_context(tc.tile_pool(name="mp", bufs=2))
            sm_pool = ph.enter_context(tc.tile_pool(name="smp", bufs=2))
            p_pool = ph.enter_context(tc.tile_pool(name="pp", bufs=2))
            pt_pool = ph.enter_context(tc.tile_pool(name="ptp", bufs=1))
            o_pool = ph.enter_context(tc.tile_pool(name="op", bufs=2))
            small_pool = ph.enter_context(tc.tile_pool(name="smallp", bufs=2))
            qk_ps = ph.enter_context(tc.tile_pool(name="qkps", bufs=2, space="PSUM"))
            pt_ps = ph.enter_context(tc.tile_pool(name="ptps", bufs=2, space="PSUM"))
            o_ps_pool = ph.enter_context(tc.tile_pool(name="ops", bufs=2, space="PSUM"))

            for s in range(8):
                n = NS[s]
                kw = n * P
                nchunks = (kw + 511) // 512
                qts = [qts_pool.tile([P, P], F32R, name=f"qts{e}", tag=f"qts{e}") for e in range(DC)]
                for e in range(DC):
                    nc.sync.dma_start(qts[e][:], qt_dram[e * P : (e + 1) * P, s * P : (s + 1) * P])
                mt = m_pool.tile([P, 512], F32, name="mt", tag="mt")
                last_nw = kw - (nchunks - 1) * 512
                nc.sync.dma_start(mt[:, :last_nw], msk[s * P : (s + 1) * P, :last_nw])
                pb = p_pool.tile([P, S], F32, name="pb", tag="pb")
                lparts = small_pool.tile([P, 4], F32, name="lparts", tag="lparts")
                for c in range(nchunks):
                    nw = min(512, kw - c * 512)
                    ps = qk_ps.tile([P, 512], F32, name="qk_t", tag="qk_t")
                    for e in range(DC):
                        nc.tensor.matmul(
                            ps[:, :nw],
                            lhsT=qts[e][:],
                            rhs=KT[e][:, c * 512 : c * 512 + nw],
                            start=(e == 0),
                            stop=(e == DC - 1),
                        )
                    if c == nchunks - 1:
                        sm = sm_pool.tile([P, 512], F32, name="sm", tag="sm")
                        nc.vector.tensor_add(sm[:, :nw], ps[:, :nw], mt[:, :nw])
                        src = sm[:, :nw]
                    else:
                        src = ps[:, :nw]
                    nc.scalar.activation(
                        pb[:, c * 512 : c * 512 + nw].bitcast(F32),
                        src,
                        mybir.ActivationFunctionType.Exp,
                        accum_out=lparts[:, c : c + 1],
                    )
                lsum = small_pool.tile([P, 1], F32, name="lsum", tag="lsum")
                nc.vector.reduce_sum(lsum[:], lparts[:, :nchunks], axis=mybir.AxisListType.X)
                pt = pt_pool.tile([P, S], F32R, name="ptt", tag="ptt")
                for j in range(n):
                    tps = pt_ps.tile([P, P], F32, name="tps_t", tag="tps_t")
                    nc.tensor.transpose(tps[:], pb[:, j * P : (j + 1) * P], ident[:])
                    nc.vector.tensor_copy(pt[:, j * P : (j + 1) * P], tps[:])
                rl = small_pool.tile([P, 1], F32, name="rl", tag="rl")
                nc.vector.reciprocal(rl[:], lsum[:])
                ot = o_pool.tile([P, D], F32, name="ot", tag="ot")
                for ec in range(2):
                    ops = o_ps_pool.tile([P, 512], F32, name="o_t", tag="o_t")
                    for j in range(n):
                        nc.tensor.matmul(
                            ops[:],
                            lhsT=pt[:, j * P : (j + 1) * P],
                            rhs=V[j][:, ec * 512 : (ec + 1) * 512],
                            start=(j == 0),
                            stop=(j == n - 1),
                        )
                    nc.vector.tensor_scalar_mul(ot[:, ec * 512 : (ec + 1) * 512], ops[:], rl[:])
                    nc.sync.dma_start(
                        out_d[s * P : (s + 1) * P, ec * 512 : (ec + 1) * 512],
                        ot[:, ec * 512 : (ec + 1) * 512],
                    )


def _get_nc():
    if "nc" not in _compiled:
        if MODE == "cc":
            _compiled["nc"] = _build_bf16_cc()
        elif MODE == "bf16":
            _compiled["nc"] = _build_bf16()
        else:
            _compiled["nc"] = _build_f32r()
    return _compiled["nc"]


def kernel(x, Wq, Wk, Wv):
    x = np.ascontiguousarray(np.asarray(x, dtype=np.float32))
    Wq = np.ascontiguousarray(np.asarray(Wq, dtype=np.float32))
    Wk = np.ascontiguousarray(np.asarray(Wk, dtype=np.float32))
    Wv = np.ascontiguousarray(np.asarray(Wv, dtype=np.float32))

    nc = _get_nc()
    if MODE == "bf16":
        in_dt = ml_dtypes.bfloat16
    else:
        in_dt = np.float32

    karr = np.arange(S)
    Wq_c = np.ascontiguousarray(Wq.astype(in_dt))
    Wk_c = np.ascontiguousarray(Wk.astype(in_dt))
    Wv_c = np.ascontiguousarray(Wv.astype(in_dt))
    in_maps = []
    for c in range(8):
        b, par = c // 2, c % 2
        tiles = A_TILES if par == 0 else B_TILES
        xb = x[b]
        xq = np.concatenate([xb[t * P : (t + 1) * P] for t in tiles], axis=0)
        xqT_np = np.ascontiguousarray((xq.T * np.float32(1.0 / 32.0)).astype(in_dt))
        xT_np = np.ascontiguousarray(xb.T.astype(in_dt))
        qg = np.concatenate([np.arange(t * P, (t + 1) * P) for t in tiles])
        # mask input: per slot, only the last 512-chunk of its key range
        m = np.zeros((1024, 512), np.float32)
        for s_i in range(8):
            kw = NS[s_i] * P
            nchunks = (kw + 511) // 512
            c0 = (nchunks - 1) * 512
            nw = kw - c0
            rows = qg[s_i * P : (s_i + 1) * P]
            m[s_i * P : (s_i + 1) * P, :nw] = np.where(
                karr[c0:kw][None, :] <= rows[:, None], np.float32(0.0), np.float32(MASK_VAL)
            )
        im = {
            "xqT": xqT_np,
            "wq": Wq_c,
            "wk": Wk_c,
            "wv": Wv_c,
            "msk": np.ascontiguousarray(m),
        }
        if MODE == "cc":
            im["xTh"] = np.ascontiguousarray(xT_np[:, par * 1024 : (par + 1) * 1024])
        else:
            im["xT"] = xT_np
        in_maps.append(im)

    trace = os.environ.get("BASS_KERNEL_TRACE", "0") == "1"
    res = run_bass_kernel_spmd(nc, in_maps, core_ids=list(range(8)), trace=trace)
    if trace:
        print(f"HW exec time: {res.exec_time_ns} ns")
        if res.instructions_and_trace is not None:
            print(f"trace: {res.instructions_and_trace[1]}")

    out = np.empty((B, S, D), np.float32)
    for c in range(8):
        b, par = c // 2, c % 2
        tiles = A_TILES if par == 0 else B_TILES
        o = res.results[c]["out"]
        for s_i, t in enumerate(tiles):
            out[b, t * P : (t + 1) * P] = o[s_i * P : (s_i + 1) * P]
    return out


# revision 20
# speedup vs baseline: 1.0457x; 1.0457x over previous
"""Causal self-attention (B=4, S=2048, D=1024, single head) on 8 TRN2 cores.

Sharding: data-parallel over batch (4 batches x 2 cores). The two cores of a
batch split the 16 query tiles (128 rows each) so both get exactly equal
causal work: core A takes tiles {0,2,4,6,9,11,13,15}, core B the complement.
The shared SPMD program processes NS[s] key-tiles for slot s (max over the
two variants); which global query tile a slot holds is carried by the input
data (host-side slicing + an additive causal mask input for the diagonal
512-chunk of each slot; earlier chunks are fully valid for both variants).

Per core: one pass over x^T strips projects Q^T, K^T (e-major) and V
(k-major), all resident in SBUF. Then per query-tile slot: scores on PE,
exp straight out of PSUM on ACT with per-chunk accumulated row sums (the
diagonal chunk takes a DVE mask-add first), PE-transpose of the
probabilities, PV accumulation on PE, normalize by the reciprocal row-sum.

Modes (BASS_KERNEL_MODE env, default "bf16"):
  "bf16" — inputs cast to bf16 host-side; ~230us, rel err 6.0e-3.
  "f32r" — fp32 storage, reduced-precision PE multiply; ~301us, rel err 3.0e-4.
  "cc"   — bf16 + intra-pair AllGather K/V dedup. DO NOT USE: collectives
           hang under the axon/PJRT execution path in this container.
"""

import os
from contextlib import ExitStack

import ml_dtypes
import numpy as np

import concourse.bacc as bacc
import concourse.mybir as mybir
import concourse.tile as tile
from concourse.bass_utils import run_bass_kernel_spmd

B, S, D = 4, 2048, 1024
P = 128
DC = D // P  # 8 contraction chunks
A_TILES = [0, 2, 4, 6, 9, 11, 13, 15]
B_TILES = [1, 3, 5, 7, 8, 10, 12, 14]
NS = [2, 4, 6, 8, 10, 12, 14, 16]  # key-tiles (128 keys) processed per slot
MASK_VAL = -60.0

F32 = mybir.dt.float32
F32R = mybir.dt.float32r
BF16 = mybir.dt.bfloat16

MODE = os.environ.get("BASS_KERNEL_MODE", "bf16")

_compiled = {}


def _build_bf16():
    nc = bacc.Bacc("TRN2", target_bir_lowering=False, debug=False)
    xqT = nc.dram_tensor("xqT", [D, 1024], BF16, kind="ExternalInput").ap()
    xT = nc.dram_tensor("xT", [D, S], BF16, kind="ExternalInput").ap()
    wq = nc.dram_tensor("wq", [D, D], BF16, kind="ExternalInput").ap()
    wk = nc.dram_tensor("wk", [D, D], BF16, kind="ExternalInput").ap()
    wv = nc.dram_tensor("wv", [D, D], BF16, kind="ExternalInput").ap()
    msk = nc.dram_tensor("msk", [1024, 512], F32, kind="ExternalInput").ap()
    out_d = nc.dram_tensor("out", [1024, D], F32, kind="ExternalOutput").ap()

    with tile.TileContext(nc) as tc:
        _body_bf16(tc, xqT, xT, wq, wk, wv, msk, out_d)
    nc.compile()
    return nc


def _body_bf16(tc, xqT, xT, wq, wk, wv, msk, out_d):
    nc = tc.nc
    with ExitStack() as top:
        const_pool = top.enter_context(tc.tile_pool(name="cst", bufs=1))
        ident = const_pool.tile([P, P], BF16, name="ident", tag="ident")
        nc.gpsimd.memset(ident[:], 1.0)
        nc.gpsimd.affine_select(
            out=ident[:],
            in_=ident[:],
            compare_op=mybir.AluOpType.is_equal,
            fill=0.0,
            base=0,
            pattern=[[-1, P]],
            channel_multiplier=1,
        )

        # resident tensors
        res_pool = top.enter_context(tc.tile_pool(name="res", bufs=1))
        QT = [res_pool.tile([P, 1024], BF16, name=f"qt{e}", tag=f"qt{e}") for e in range(DC)]
        KT = [res_pool.tile([P, S], BF16, name=f"kt{e}", tag=f"kt{e}") for e in range(DC)]
        V = [res_pool.tile([P, D], BF16, name=f"v{k}", tag=f"v{k}") for k in range(S // P)]

        # weights resident (bf16: 3 * 16KB/partition)
        w_pool = top.enter_context(tc.tile_pool(name="wp", bufs=1))
        wq_t = [w_pool.tile([P, D], BF16, name=f"wqt{d}", tag=f"wqt{d}") for d in range(DC)]
        wk_t = [w_pool.tile([P, D], BF16, name=f"wkt{d}", tag=f"wkt{d}") for d in range(DC)]
        wv_t = [w_pool.tile([P, D], BF16, name=f"wvt{d}", tag=f"wvt{d}") for d in range(DC)]

        # one PSUM pool for the whole kernel (8 banks: 2+2+2+2)
        psum = top.enter_context(tc.tile_pool(name="psum", bufs=2, space="PSUM"))
        # streaming x strips
        xs_pool = top.enter_context(tc.tile_pool(name="xsp", bufs=2))

        # ---------------- K + V projections (single strip pass) ----------------
        # V runs first within each strip: its first matmul group is gated on
        # only strip0 + the ec0 half of Wv (2MB), vs 3MB for a W-stationary
        # first group — PE starts earlier. Q runs last (attention needs Q^T
        # no sooner than the end of the projections anyway).
        xs0 = [xs_pool.tile([P, 512], BF16, name=f"xs{d}", tag=f"xs{d}") for d in range(DC)]
        for d in range(DC):
            nc.sync.dma_start(xs0[d][:], xT[d * P : (d + 1) * P, 0:512])
        for ec in range(2):
            for d in range(DC):
                nc.sync.dma_start(
                    wv_t[d][:, ec * 512 : (ec + 1) * 512],
                    wv[d * P : (d + 1) * P, ec * 512 : (ec + 1) * 512],
                )
        for d in range(DC):
            nc.sync.dma_start(wk_t[d][:], wk[d * P : (d + 1) * P, :])
        for kc in range(4):
            if kc == 0:
                xs = xs0
            else:
                xs = [xs_pool.tile([P, 512], BF16, name=f"xs{d}", tag=f"xs{d}") for d in range(DC)]
                for d in range(DC):
                    nc.sync.dma_start(xs[d][:], xT[d * P : (d + 1) * P, kc * 512 : (kc + 1) * 512])
            for j in range(4):
                kt_idx = kc * 4 + j
                for ec in range(2):
                    ps = psum.tile([P, 512], F32, name="pps", tag="pps")
                    for d in range(DC):
                        nc.tensor.matmul(
                            ps[:],
                            lhsT=xs[d][:, j * P : (j + 1) * P],
                            rhs=wv_t[d][:, ec * 512 : (ec + 1) * 512],
                            start=(d == 0),
                            stop=(d == DC - 1),
                        )
                    nc.scalar.copy(V[kt_idx][:, ec * 512 : (ec + 1) * 512], ps[:])
            for e in range(DC):
                ps = psum.tile([P, 512], F32, name="pps", tag="pps")
                for d in range(DC):
                    nc.tensor.matmul(
                        ps[:],
                        lhsT=wk_t[d][:, e * P : (e + 1) * P],
                        rhs=xs[d][:],
                        start=(d == 0),
                        stop=(d == DC - 1),
                    )
                nc.scalar.copy(KT[e][:, kc * 512 : (kc + 1) * 512], ps[:])

        # ---------------- Q projection (last) ----------------
        for d in range(DC):
            nc.sync.dma_start(wq_t[d][:], wq[d * P : (d + 1) * P, :])
        for qc in range(2):
            xs = [xs_pool.tile([P, 512], BF16, name=f"xs{d}", tag=f"xs{d}") for d in range(DC)]
            for d in range(DC):
                nc.sync.dma_start(xs[d][:], xqT[d * P : (d + 1) * P, qc * 512 : (qc + 1) * 512])
            for e in range(DC):
                ps = psum.tile([P, 512], F32, name="pps", tag="pps")
                for d in range(DC):
                    nc.tensor.matmul(
                        ps[:],
                        lhsT=wq_t[d][:, e * P : (e + 1) * P],
                        rhs=xs[d][:],
                        start=(d == 0),
                        stop=(d == DC - 1),
                    )
                nc.scalar.copy(QT[e][:, qc * 512 : (qc + 1) * 512], ps[:])

        # ---------------- attention over slots ----------------
        with ExitStack() as ph:
            m_pool = ph.enter_context(tc.tile_pool(name="mp", bufs=2))
            sm_pool = ph.enter_context(tc.tile_pool(name="smp", bufs=2))
            p_pool = ph.enter_context(tc.tile_pool(name="pp", bufs=2))
            pt_pool = ph.enter_context(tc.tile_pool(name="ptp", bufs=2))
            o_pool = ph.enter_context(tc.tile_pool(name="op", bufs=2))
            small_pool = ph.enter_context(tc.tile_pool(name="smallp", bufs=2))

            for s in range(8):
                n = NS[s]
                kw = n * P
                nchunks = (kw + 511) // 512
                mt = m_pool.tile([P, 512], F32, name="mt", tag="mt")
                last_nw = kw - (nchunks - 1) * 512
                nc.sync.dma_start(mt[:, :last_nw], msk[s * P : (s + 1) * P, :last_nw])
                pb = p_pool.tile([P, S], BF16, name="pb", tag="pb")
                lparts = small_pool.tile([P, 4], F32, name="lparts", tag="lparts")
                for c in range(nchunks):
                    nw = min(512, kw - c * 512)
                    ps = psum.tile([P, 512], F32, name="qk_t", tag="qk_t")
                    for e in range(DC):
                        nc.tensor.matmul(
                            ps[:, :nw],
                            lhsT=QT[e][:, s * P : (s + 1) * P],
                            rhs=KT[e][:, c * 512 : c * 512 + nw],
                            start=(e == 0),
                            stop=(e == DC - 1),
                        )
                    if c == nchunks - 1:
                        sm = sm_pool.tile([P, 512], F32, name="sm", tag="sm")
                        nc.vector.tensor_add(sm[:, :nw], ps[:, :nw], mt[:, :nw])
                        src = sm[:, :nw]
                    else:
                        src = ps[:, :nw]
                    nc.scalar.activation(
                        pb[:, c * 512 : c * 512 + nw],
                        src,
                        mybir.ActivationFunctionType.Exp,
                        accum_out=lparts[:, c : c + 1],
                    )
                lsum = small_pool.tile([P, 1], F32, name="lsum", tag="lsum")
                nc.vector.reduce_sum(lsum[:], lparts[:, :nchunks], axis=mybir.AxisListType.X)
                pt = pt_pool.tile([P, S], BF16, name="ptt", tag="ptt")
                for j in range(n):
                    tps = psum.tile([P, P], BF16, name="tps_t", tag="tps_t")
                    nc.tensor.transpose(tps[:], pb[:, j * P : (j + 1) * P], ident[:])
                    nc.vector.tensor_copy(pt[:, j * P : (j + 1) * P], tps[:])
                rl = small_pool.tile([P, 1], F32, name="rl", tag="rl")
                nc.vector.reciprocal(rl[:], lsum[:])
                ot = o_pool.tile([P, D], F32, name="ot", tag="ot")
                for ec in range(2):
                    ops = psum.tile([P, 512], F32, name="o_t", tag="o_t")
                    for j in range(n):
                        nc.tensor.matmul(
                            ops[:],
                            lhsT=pt[:, j * P : (j + 1) * P],
                            rhs=V[j][:, ec * 512 : (ec + 1) * 512],
                            start=(j == 0),
                            stop=(j == n - 1),
                        )
                    nc.vector.tensor_scalar_mul(ot[:, ec * 512 : (ec + 1) * 512], ops[:], rl[:])
                    nc.sync.dma_start(
                        out_d[s * P : (s + 1) * P, ec * 512 : (ec + 1) * 512],
                        ot[:, ec * 512 : (ec + 1) * 512],
                    )


# ---------------------------------------------------------------------------
# bf16 + collective K/V projection dedup: each core projects only its key-half
# of K^T and V; an intra-pair AllGather (ranks {2b, 2b+1}) rebuilds the full
# tensors. Saves 256 of 640 projection matmuls per core.
# ---------------------------------------------------------------------------


def _build_bf16_cc():
    nc = bacc.Bacc("TRN2", target_bir_lowering=False, debug=False)
    xqT = nc.dram_tensor("xqT", [D, 1024], BF16, kind="ExternalInput").ap()
    xTh = nc.dram_tensor("xTh", [D, 1024], BF16, kind="ExternalInput").ap()
    wq = nc.dram_tensor("wq", [D, D], BF16, kind="ExternalInput").ap()
    wk = nc.dram_tensor("wk", [D, D], BF16, kind="ExternalInput").ap()
    wv = nc.dram_tensor("wv", [D, D], BF16, kind="ExternalInput").ap()
    msk = nc.dram_tensor("msk", [1024, 512], F32, kind="ExternalInput").ap()
    out_d = nc.dram_tensor("out", [1024, D], F32, kind="ExternalOutput").ap()

    with tile.TileContext(nc) as tc:
        _body_bf16_cc(tc, xqT, xTh, wq, wk, wv, msk, out_d)
    nc.compile()
    return nc


def _body_bf16_cc(tc, xqT, xTh, wq, wk, wv, msk, out_d):
    nc = tc.nc
    RG = [[0, 1], [2, 3], [4, 5], [6, 7]]
    with ExitStack() as top:
        const_pool = top.enter_context(tc.tile_pool(name="cst", bufs=1))
        ident = const_pool.tile([P, P], BF16, name="ident", tag="ident")
        nc.gpsimd.memset(ident[:], 1.0)
        nc.gpsimd.affine_select(
            out=ident[:],
            in_=ident[:],
            compare_op=mybir.AluOpType.is_equal,
            fill=0.0,
            base=0,
            pattern=[[-1, P]],
            channel_multiplier=1,
        )

        res_pool = top.enter_context(tc.tile_pool(name="res", bufs=1))
        QT = [res_pool.tile([P, 1024], BF16, name=f"qt{e}", tag=f"qt{e}") for e in range(DC)]
        KT = [res_pool.tile([P, S], BF16, name=f"kt{e}", tag=f"kt{e}") for e in range(DC)]
        V = [res_pool.tile([P, D], BF16, name=f"v{k}", tag=f"v{k}") for k in range(S // P)]

        w_pool = top.enter_context(tc.tile_pool(name="wp", bufs=1))
        wq_t = [w_pool.tile([P, D], BF16, name=f"wqt{d}", tag=f"wqt{d}") for d in range(DC)]
        wk_t = [w_pool.tile([P, D], BF16, name=f"wkt{d}", tag=f"wkt{d}") for d in range(DC)]
        wv_t = [w_pool.tile([P, D], BF16, name=f"wvt{d}", tag=f"wvt{d}") for d in range(DC)]
        for d in range(DC):
            nc.sync.dma_start(wk_t[d][:], wk[d * P : (d + 1) * P, :])
            nc.sync.dma_start(wv_t[d][:], wv[d * P : (d + 1) * P, :])

        dram_pool = top.enter_context(tc.tile_pool(name="ccb", bufs=1, space="DRAM"))
        k_in = dram_pool.tile([1024, 1024], BF16, name="k_in", tag="k_in")
        k_out = dram_pool.tile([2048, 1024], BF16, name="k_out", tag="k_out")
        v_in = dram_pool.tile([1024, 1024], BF16, name="v_in", tag="v_in")
        v_out = dram_pool.tile([2048, 1024], BF16, name="v_out", tag="v_out")

        psum = top.enter_context(tc.tile_pool(name="psum", bufs=2, space="PSUM"))
        xs_pool = top.enter_context(tc.tile_pool(name="xsp", bufs=2))
        stage_pool = top.enter_context(tc.tile_pool(name="stg", bufs=4))

        # ---------------- K half projection -> k_in ----------------
        for kc in range(2):
            xs = [xs_pool.tile([P, 512], BF16, name=f"xs{d}", tag=f"xs{d}") for d in range(DC)]
            for d in range(DC):
                nc.sync.dma_start(xs[d][:], xTh[d * P : (d + 1) * P, kc * 512 : (kc + 1) * 512])
            for e in range(DC):
                ps = psum.tile([P, 512], F32, name="pps", tag="pps")
                for d in range(DC):
                    nc.tensor.matmul(
                        ps[:],
                        lhsT=wk_t[d][:, e * P : (e + 1) * P],
                        rhs=xs[d][:],
                        start=(d == 0),
                        stop=(d == DC - 1),
                    )
                sb = stage_pool.tile([P, 512], BF16, name="ksb", tag="ksb")
                nc.scalar.copy(sb[:], ps[:])
                nc.sync.dma_start(k_in[e * P : (e + 1) * P, kc * 512 : (kc + 1) * 512], sb[:])
        nc.gpsimd.collective_compute(
            "AllGather",
            mybir.AluOpType.bypass,
            replica_groups=RG,
            ins=[k_in[:]],
            outs=[k_out[:]],
        )

        # ---------------- V half projection -> v_in ----------------
        for kc in range(2):
            xs = [xs_pool.tile([P, 512], BF16, name=f"xs{d}", tag=f"xs{d}") for d in range(DC)]
            for d in range(DC):
                nc.sync.dma_start(xs[d][:], xTh[d * P : (d + 1) * P, kc * 512 : (kc + 1) * 512])
            for j in range(4):
                ktl = kc * 4 + j
                for ec in range(2):
                    ps = psum.tile([P, 512], F32, name="pps", tag="pps")
                    for d in range(DC):
                        nc.tensor.matmul(
                            ps[:],
                            lhsT=xs[d][:, j * P : (j + 1) * P],
                            rhs=wv_t[d][:, ec * 512 : (ec + 1) * 512],
                            start=(d == 0),
                            stop=(d == DC - 1),
                        )
                    sb = stage_pool.tile([P, 512], BF16, name="vsb", tag="vsb")
                    nc.scalar.copy(sb[:], ps[:])
                    nc.sync.dma_start(v_in[ktl * P : (ktl + 1) * P, ec * 512 : (ec + 1) * 512], sb[:])
        nc.gpsimd.collective_compute(
            "AllGather",
            mybir.AluOpType.bypass,
            replica_groups=RG,
            ins=[v_in[:]],
            outs=[v_out[:]],
        )

        # ---------------- Q projection (local, no comm) ----------------
        for d in range(DC):
            nc.sync.dma_start(wq_t[d][:], wq[d * P : (d + 1) * P, :])
        for qc in range(2):
            xs = [xs_pool.tile([P, 512], BF16, name=f"xs{d}", tag=f"xs{d}") for d in range(DC)]
            for d in range(DC):
                nc.sync.dma_start(xs[d][:], xqT[d * P : (d + 1) * P, qc * 512 : (qc + 1) * 512])
            for e in range(DC):
                ps = psum.tile([P, 512], F32, name="pps", tag="pps")
                for d in range(DC):
                    nc.tensor.matmul(
                        ps[:],
                        lhsT=wq_t[d][:, e * P : (e + 1) * P],
                        rhs=xs[d][:],
                        start=(d == 0),
                        stop=(d == DC - 1),
                    )
                nc.scalar.copy(QT[e][:, qc * 512 : (qc + 1) * 512], ps[:])

        # ---------------- gather results -> SBUF residents ----------------
        for e in range(DC):
            nc.sync.dma_start(KT[e][:, 0:1024], k_out[e * P : (e + 1) * P, :])
            nc.sync.dma_start(KT[e][:, 1024:2048], k_out[1024 + e * P : 1024 + (e + 1) * P, :])
        for kt in range(S // P):
            nc.sync.dma_start(V[kt][:], v_out[kt * P : (kt + 1) * P, :])

        # ---------------- attention over slots ----------------
        with ExitStack() as ph:
            m_pool = ph.enter_context(tc.tile_pool(name="mp", bufs=2))
            sm_pool = ph.enter_context(tc.tile_pool(name="smp", bufs=2))
            p_pool = ph.enter_context(tc.tile_pool(name="pp", bufs=2))
            pt_pool = ph.enter_context(tc.tile_pool(name="ptp", bufs=2))
            o_pool = ph.enter_context(tc.tile_pool(name="op", bufs=2))
            small_pool = ph.enter_context(tc.tile_pool(name="smallp", bufs=2))

            for s in range(8):
                n = NS[s]
                kw = n * P
                nchunks = (kw + 511) // 512
                mt = m_pool.tile([P, 512], F32, name="mt", tag="mt")
                last_nw = kw - (nchunks - 1) * 512
                nc.sync.dma_start(mt[:, :last_nw], msk[s * P : (s + 1) * P, :last_nw])
                pb = p_pool.tile([P, S], BF16, name="pb", tag="pb")
                lparts = small_pool.tile([P, 4], F32, name="lparts", tag="lparts")
                for c in range(nchunks):
                    nw = min(512, kw - c * 512)
                    ps = psum.tile([P, 512], F32, name="qk_t", tag="qk_t")
                    for e in range(DC):
                        nc.tensor.matmul(
                            ps[:, :nw],
                            lhsT=QT[e][:, s * P : (s + 1) * P],
                            rhs=KT[e][:, c * 512 : c * 512 + nw],
                            start=(e == 0),
                            stop=(e == DC - 1),
                        )
                    if c == nchunks - 1:
                        sm = sm_pool.tile([P, 512], F32, name="sm", tag="sm")
                        nc.vector.tensor_add(sm[:, :nw], ps[:, :nw], mt[:, :nw])
                        src = sm[:, :nw]
                    else:
                        src = ps[:, :nw]
                    nc.scalar.activation(
                        pb[:, c * 512 : c * 512 + nw],
                        src,
                        mybir.ActivationFunctionType.Exp,
                        accum_out=lparts[:, c : c + 1],
                    )
                lsum = small_pool.tile([P, 1], F32, name="lsum", tag="lsum")
                nc.vector.reduce_sum(lsum[:], lparts[:, :nchunks], axis=mybir.AxisListType.X)
                pt = pt_pool.tile([P, S], BF16, name="ptt", tag="ptt")
                for j in range(n):
                    tps = psum.tile([P, P], BF16, name="tps_t", tag="tps_t")
                    nc.tensor.transpose(tps[:], pb[:, j * P : (j + 1) * P], ident[:])
                    nc.vector.tensor_copy(pt[:, j * P : (j + 1) * P], tps[:])
                rl = small_pool.tile([P, 1], F32, name="rl", tag="rl")
                nc.vector.reciprocal(rl[:], lsum[:])
                ot = o_pool.tile([P, D], F32, name="ot", tag="ot")
                for ec in range(2):
                    ops = psum.tile([P, 512], F32, name="o_t", tag="o_t")
                    for j in range(n):
                        nc.tensor.matmul(
                            ops[:],
                            lhsT=pt[:, j * P : (j + 1) * P],
                            rhs=V[j][:, ec * 512 : (ec + 1) * 512],
                            start=(j == 0),
                            stop=(j == n - 1),
                        )
                    nc.vector.tensor_scalar_mul(ot[:, ec * 512 : (ec + 1) * 512], ops[:], rl[:])
                    nc.sync.dma_start(
                        out_d[s * P : (s + 1) * P, ec * 512 : (ec + 1) * 512],
                        ot[:, ec * 512 : (ec + 1) * 512],
                    )


# ---------------------------------------------------------------------------
# f32r fallback (fp32 storage, reduced-precision multiply; Q^T spilled to DRAM)
# ---------------------------------------------------------------------------


def _build_f32r():
    nc = bacc.Bacc("TRN2", target_bir_lowering=False, debug=False)
    xqT = nc.dram_tensor("xqT", [D, 1024], F32R, kind="ExternalInput").ap()
    xT = nc.dram_tensor("xT", [D, S], F32R, kind="ExternalInput").ap()
    wq = nc.dram_tensor("wq", [D, D], F32R, kind="ExternalInput").ap()
    wk = nc.dram_tensor("wk", [D, D], F32R, kind="ExternalInput").ap()
    wv = nc.dram_tensor("wv", [D, D], F32R, kind="ExternalInput").ap()
    msk = nc.dram_tensor("msk", [1024, 512], F32, kind="ExternalInput").ap()
    out_d = nc.dram_tensor("out", [1024, D], F32, kind="ExternalOutput").ap()
    qt_dram = nc.dram_tensor("qt_spill", [D, 1024], F32R).ap()

    with tile.TileContext(nc) as tc:
        _body_f32r(tc, xqT, xT, wq, wk, wv, msk, out_d, qt_dram)
    nc.compile()
    return nc


def _body_f32r(tc, xqT, xT, wq, wk, wv, msk, out_d, qt_dram):
    nc = tc.nc
    with ExitStack() as top:
        const_pool = top.enter_context(tc.tile_pool(name="cst", bufs=1))
        ident = const_pool.tile([P, P], F32, name="ident", tag="ident")
        nc.gpsimd.memset(ident[:], 1.0)
        nc.gpsimd.affine_select(
            out=ident[:],
            in_=ident[:],
            compare_op=mybir.AluOpType.is_equal,
            fill=0.0,
            base=0,
            pattern=[[-1, P]],
            channel_multiplier=1,
        )

        # ---------------- Q projection: Q^T -> DRAM spill ----------------
        with ExitStack() as ph:
            xq_pool = ph.enter_context(tc.tile_pool(name="xqp", bufs=1))
            wq_pool = ph.enter_context(tc.tile_pool(name="wqp", bufs=1))
            qps_pool = ph.enter_context(tc.tile_pool(name="qps", bufs=3, space="PSUM"))
            qsb_pool = ph.enter_context(tc.tile_pool(name="qsb", bufs=3))
            xq_t = [xq_pool.tile([P, 1024], F32R, name=f"xq{d}", tag=f"xq{d}") for d in range(DC)]
            wq_t = [wq_pool.tile([P, D], F32R, name=f"wqt{d}", tag=f"wqt{d}") for d in range(DC)]
            for d in range(DC):
                nc.sync.dma_start(xq_t[d][:], xqT[d * P : (d + 1) * P, :])
                nc.sync.dma_start(wq_t[d][:], wq[d * P : (d + 1) * P, :])
            for e in range(DC):
                for qc in range(2):
                    ps = qps_pool.tile([P, 512], F32, name="qps_t", tag="qps_t")
                    for d in range(DC):
                        nc.tensor.matmul(
                            ps[:],
                            lhsT=wq_t[d][:, e * P : (e + 1) * P],
                            rhs=xq_t[d][:, qc * 512 : (qc + 1) * 512],
                            start=(d == 0),
                            stop=(d == DC - 1),
                        )
                    sb = qsb_pool.tile([P, 512], F32R, name="qsb_t", tag="qsb_t")
                    nc.scalar.copy(sb[:], ps[:])
                    nc.sync.dma_start(
                        qt_dram[e * P : (e + 1) * P, qc * 512 : (qc + 1) * 512], sb[:]
                    )

        # ---------------- K projection: K^T resident ----------------
        kt_pool = top.enter_context(tc.tile_pool(name="ktp", bufs=1))
        KT = [kt_pool.tile([P, S], F32R, name=f"kt{e}", tag=f"kt{e}") for e in range(DC)]
        with ExitStack() as ph:
            wk_pool = ph.enter_context(tc.tile_pool(name="wkp", bufs=1))
            xs_pool = ph.enter_context(tc.tile_pool(name="xsp", bufs=2))
            kps_pool = ph.enter_context(tc.tile_pool(name="kps", bufs=3, space="PSUM"))
            wk_t = [wk_pool.tile([P, D], F32R, name=f"wkt{d}", tag=f"wkt{d}") for d in range(DC)]
            for d in range(DC):
                nc.sync.dma_start(wk_t[d][:], wk[d * P : (d + 1) * P, :])
            for kc in range(4):
                xs = [xs_pool.tile([P, 512], F32R, name=f"xs{d}", tag=f"xs{d}") for d in range(DC)]
                for d in range(DC):
                    nc.sync.dma_start(xs[d][:], xT[d * P : (d + 1) * P, kc * 512 : (kc + 1) * 512])
                for e in range(DC):
                    ps = kps_pool.tile([P, 512], F32, name="kps_t", tag="kps_t")
                    for d in range(DC):
                        nc.tensor.matmul(
                            ps[:],
                            lhsT=wk_t[d][:, e * P : (e + 1) * P],
                            rhs=xs[d][:],
                            start=(d == 0),
                            stop=(d == DC - 1),
                        )
                    nc.scalar.copy(KT[e][:, kc * 512 : (kc + 1) * 512], ps[:])

        # ---------------- V projection: V resident ----------------
        v_pool = top.enter_context(tc.tile_pool(name="vp", bufs=1))
        V = [v_pool.tile([P, D], F32R, name=f"v{k}", tag=f"v{k}") for k in range(S // P)]
        with ExitStack() as ph:
            wv_pool = ph.enter_context(tc.tile_pool(name="wvp", bufs=1))
            xsv_pool = ph.enter_context(tc.tile_pool(name="xsvp", bufs=2))
            vps_pool = ph.enter_context(tc.tile_pool(name="vps", bufs=3, space="PSUM"))
            for ec in range(2):
                wv_t = [wv_pool.tile([P, 512], F32R, name=f"wvt{d}", tag=f"wvt{d}") for d in range(DC)]
                for d in range(DC):
                    nc.sync.dma_start(wv_t[d][:], wv[d * P : (d + 1) * P, ec * 512 : (ec + 1) * 512])
                for kc in range(4):
                    xs = [xsv_pool.tile([P, 512], F32R, name=f"xsv{d}", tag=f"xsv{d}") for d in range(DC)]
                    for d in range(DC):
                        nc.sync.dma_start(
                            xs[d][:], xT[d * P : (d + 1) * P, kc * 512 : (kc + 1) * 512]
                        )
                    for j in range(4):
                        kt_idx = kc * 4 + j
                        ps = vps_pool.tile([P, 512], F32, name="vps_t", tag="vps_t")
                        for d in range(DC):
                            nc.tensor.matmul(
                                ps[:],
                                lhsT=xs[d][:, j * P : (j + 1) * P],
                                rhs=wv_t[d][:],
                                start=(d == 0),
                                stop=(d == DC - 1),
                            )
                        nc.scalar.copy(V[kt_idx][:, ec * 512 : (ec + 1) * 512], ps[:])

        # ---------------- attention over slots ----------------
        with ExitStack() as ph:
            qts_pool = ph.enter_context(tc.tile_pool(name="qtsp", bufs=2))
            m_pool = ph.enter_context(tc.tile_pool(name="mp", bufs=2))
            sm_pool = ph.enter_context(tc.tile_pool(name="smp", bufs=2))
            p_pool = ph.enter_context(tc.tile_pool(name="pp", bufs=2))
            pt_pool = ph.enter_context(tc.tile_pool(name="ptp", bufs=1))
            o_pool = ph.enter_context(tc.tile_pool(name="op", bufs=2))
            small_pool = ph.enter_context(tc.tile_pool(name="smallp", bufs=2))
            qk_ps = ph.enter_context(tc.tile_pool(name="qkps", bufs=2, space="PSUM"))
            pt_ps = ph.enter_context(tc.tile_pool(name="ptps", bufs=2, space="PSUM"))
            o_ps_pool = ph.enter_context(tc.tile_pool(name="ops", bufs=2, space="PSUM"))

            for s in range(8):
                n = NS[s]
                kw = n * P
                nchunks = (kw + 511) // 512
                qts = [qts_pool.tile([P, P], F32R, name=f"qts{e}", tag=f"qts{e}") for e in range(DC)]
                for e in range(DC):
                    nc.sync.dma_start(qts[e][:], qt_dram[e * P : (e + 1) * P, s * P : (s + 1) * P])
                mt = m_pool.tile([P, 512], F32, name="mt", tag="mt")
                last_nw = kw - (nchunks - 1) * 512
                nc.sync.dma_start(mt[:, :last_nw], msk[s * P : (s + 1) * P, :last_nw])
                pb = p_pool.tile([P, S], F32, name="pb", tag="pb")
                lparts = small_pool.tile([P, 4], F32, name="lparts", tag="lparts")
                for c in range(nchunks):
                    nw = min(512, kw - c * 512)
                    ps = qk_ps.tile([P, 512], F32, name="qk_t", tag="qk_t")
                    for e in range(DC):
                        nc.tensor.matmul(
                            ps[:, :nw],
                            lhsT=qts[e][:],
                            rhs=KT[e][:, c * 512 : c * 512 + nw],
                            start=(e == 0),
                            stop=(e == DC - 1),
                        )
                    if c == nchunks - 1:
                        sm = sm_pool.tile([P, 512], F32, name="sm", tag="sm")
                        nc.vector.tensor_add(sm[:, :nw], ps[:, :nw], mt[:, :nw])
                        src = sm[:, :nw]
                    else:
                        src = ps[:, :nw]
                    nc.scalar.activation(
                        pb[:, c * 512 : c * 512 + nw].bitcast(F32),
                        src,
                        mybir.ActivationFunctionType.Exp,
                        accum_out=lparts[:, c : c + 1],
                    )
                lsum = small_pool.tile([P, 1], F32, name="lsum", tag="lsum")
                nc.vector.reduce_sum(lsum[:], lparts[:, :nchunks], axis=mybir.AxisListType.X)
                pt = pt_pool.tile([P, S], F32R, name="ptt", tag="ptt")
                for j in range(n):
                    tps = pt_ps.tile([P, P], F32, name="tps_t", tag="tps_t")
                    nc.tensor.transpose(tps[:], pb[:, j * P : (j + 1) * P], ident[:])
                    nc.vector.tensor_copy(pt[:, j * P : (j + 1) * P], tps[:])
                rl = small_pool.tile([P, 1], F32, name="rl", tag="rl")
                nc.vector.reciprocal(rl[:], lsum[:])
                ot = o_pool.tile([P, D], F32, name="ot", tag="ot")
                for ec in range(2):
                    ops = o_ps_pool.tile([P, 512], F32, name="o_t", tag="o_t")
                    for j in range(n):
                        nc.tensor.matmul(
                            ops[:],
                            lhsT=pt[:, j * P : (j + 1) * P],
                            rhs=V[j][:, ec * 512 : (ec + 1) * 512],
                            start=(j == 0),
                            stop=(j == n - 1),
                        )
                    nc.vector.tensor_scalar_mul(ot[:, ec * 512 : (ec + 1) * 512], ops[:], rl[:])
                    nc.sync.dma_start(
                        out_d[s * P : (s + 1) * P, ec * 512 : (ec + 1) * 512],
                        ot[:, ec * 512 : (ec + 1) * 512],
                    )


def _get_nc():
    if "nc" not in _compiled:
        if MODE == "cc":
            _compiled["nc"] = _build_bf16_cc()
        elif MODE == "bf16":
            _compiled["nc"] = _build_bf16()
        else:
            _compiled["nc"] = _build_f32r()
    return _compiled["nc"]


def kernel(x, Wq, Wk, Wv):
    x = np.ascontiguousarray(np.asarray(x, dtype=np.float32))
    Wq = np.ascontiguousarray(np.asarray(Wq, dtype=np.float32))
    Wk = np.ascontiguousarray(np.asarray(Wk, dtype=np.float32))
    Wv = np.ascontiguousarray(np.asarray(Wv, dtype=np.float32))

    nc = _get_nc()
    if MODE == "bf16":
        in_dt = ml_dtypes.bfloat16
    else:
        in_dt = np.float32

    karr = np.arange(S)
    Wq_c = np.ascontiguousarray(Wq.astype(in_dt))
    Wk_c = np.ascontiguousarray(Wk.astype(in_dt))
    Wv_c = np.ascontiguousarray(Wv.astype(in_dt))
    in_maps = []
    for c in range(8):
        b, par = c // 2, c % 2
        tiles = A_TILES if par == 0 else B_TILES
        xb = x[b]
        xq = np.concatenate([xb[t * P : (t + 1) * P] for t in tiles], axis=0)
        xqT_np = np.ascontiguousarray((xq.T * np.float32(1.0 / 32.0)).astype(in_dt))
        xT_np = np.ascontiguousarray(xb.T.astype(in_dt))
        qg = np.concatenate([np.arange(t * P, (t + 1) * P) for t in tiles])
        # mask input: per slot, only the last 512-chunk of its key range
        m = np.zeros((1024, 512), np.float32)
        for s_i in range(8):
            kw = NS[s_i] * P
            nchunks = (kw + 511) // 512
            c0 = (nchunks - 1) * 512
            nw = kw - c0
            rows = qg[s_i * P : (s_i + 1) * P]
            m[s_i * P : (s_i + 1) * P, :nw] = np.where(
                karr[c0:kw][None, :] <= rows[:, None], np.float32(0.0), np.float32(MASK_VAL)
            )
        im = {
            "xqT": xqT_np,
            "wq": Wq_c,
            "wk": Wk_c,
            "wv": Wv_c,
            "msk": np.ascontiguousarray(m),
        }
        if MODE == "cc":
            im["xTh"] = np.ascontiguousarray(xT_np[:, par * 1024 : (par + 1) * 1024])
        else:
            im["xT"] = xT_np
        in_maps.append(im)

    trace = os.environ.get("BASS_KERNEL_TRACE", "0") == "1"
    res = run_bass_kernel_spmd(nc, in_maps, core_ids=list(range(8)), trace=trace)
    if trace:
        print(f"HW exec time: {res.exec_time_ns} ns")
        if res.instructions_and_trace is not None:
            print(f"trace: {res.instructions_and_trace[1]}")

    out = np.empty((B, S, D), np.float32)
    for c in range(8):
        b, par = c // 2, c % 2
        tiles = A_TILES if par == 0 else B_TILES
        o = res.results[c]["out"]
        for s_i, t in enumerate(tiles):
            out[b, t * P : (t + 1) * P] = o[s_i * P : (s_i + 1) * P]
    return out
